# revision 1
# baseline (speedup 1.0000x reference)
"""Trainium2 Bass kernel for phase-field fracture FEM energy (gnn_message_passing).

Sharding: elements split across 8 NeuronCores (data-parallel); nodal arrays
enter element space via a uvc gather done during host-side input prep; the
three scalar energy sums are reduced per-(core, partition) on device and the
final reduction happens on host (the output-unshard step).

Device kernel per core (32768 elements = 128 partitions x 256 elems):
  - streams N/dNdx/B (bf16), uvc_el/volumes (f32)
  - einsums over nodes-per-element via tensor_tensor mult + free-axis reduce
  - fracture + Amor-split elastic energy densities, fused scale constants
  - E_irr from a node shard of (c, prev_c)
  - accumulates 3 partial sums per partition -> out [128, 4]
"""
import numpy as np
import ml_dtypes

# --- problem constants (from reference) --------------------------------------
G_C = 0.0027
L_0 = 0.015
PF_TOL = 0.01
ENERGY_SCALING = 1.0
NU = 0.3
E_MOD = 210.0
LAM = E_MOD * NU / ((1.0 + NU) * (1.0 - 2.0 * NU))
MU = E_MOD / (2.0 * (1.0 + NU))
K_MOD = LAM + 2.0 * MU / 3.0
PENALTY = G_C / L_0 * (1.0 / PF_TOL**2 - 1.0) * ENERGY_SCALING

N_NODES = 263169
N_ELEMS = 262144
NCORES = 8
P = 128
EC = N_ELEMS // NCORES          # 32768 elements per core
EPP = EC // P                   # 256 elements per partition
TE = 64                         # elements per partition per tile (tunable)
NT = EPP // TE                  # tiles
NODE_PAD = 33024                # per-core node shard (128*258), 8*33024 >= N_NODES
NODE_F = NODE_PAD // P          # 258

TRACE = False
SIM_EXEC_NS = 56247  # CoreSim cost-model predicted per-core exec (no NTFF profiling under axon)
COMPUTE = True
LOAD_BUFS = 2
SCRATCH_BUFS = 2
LAST_EXEC_NS = None  # populated only when NTFF tracing is available
_CACHE = {}


def _build_bass():
    import concourse.bacc as bacc
    import concourse.tile as tile
    from concourse import mybir

    f32 = mybir.dt.float32
    bf16 = mybir.dt.bfloat16
    Alu = mybir.AluOpType
    Act = mybir.ActivationFunctionType

    TE_ = TE
    NT_ = EPP // TE_
    nc = bacc.Bacc("TRN2")
    d_uvc = nc.dram_tensor("uvc", [P, EPP * 12], f32, kind="ExternalInput")
    d_n = nc.dram_tensor("nmat", [P, EPP * 16], bf16, kind="ExternalInput")
    d_dn = nc.dram_tensor("dmat", [P, EPP * 32], bf16, kind="ExternalInput")
    d_b = nc.dram_tensor("bmat", [P, EPP * 96], bf16, kind="ExternalInput")
    d_vol = nc.dram_tensor("vol", [P, EPP * 4], f32, kind="ExternalInput")
    d_c = nc.dram_tensor("cnd", [P, NODE_F], f32, kind="ExternalInput")
    d_pc = nc.dram_tensor("pnd", [P, NODE_F], f32, kind="ExternalInput")
    d_out = nc.dram_tensor("out", [P, 4], f32, kind="ExternalOutput")

    with tile.TileContext(nc) as tc:
        with (
            tc.tile_pool(name="loads", bufs=LOAD_BUFS) as loads,
            tc.tile_pool(name="scratch", bufs=SCRATCH_BUFS) as scratch,
            tc.tile_pool(name="acc", bufs=1) as accp,
        ):
            accE = accp.tile([P, 1], f32)
            accF = accp.tile([P, 1], f32)
            accI = accp.tile([P, 1], f32)
            nc.vector.memset(accE[:], 0.0)
            nc.vector.memset(accF[:], 0.0)
            nc.vector.memset(accI[:], 0.0)

            # ---- E_irr over the node shard ---------------------------------
            t_c = accp.tile([P, NODE_F], f32)
            t_pc = accp.tile([P, NODE_F], f32)
            nc.sync.dma_start(out=t_c[:], in_=d_c[:])
            nc.sync.dma_start(out=t_pc[:], in_=d_pc[:])
            t_d = accp.tile([P, NODE_F], f32)
            nc.vector.tensor_tensor(out=t_d[:], in0=t_pc[:], in1=t_c[:], op=Alu.subtract)
            t_r = accp.tile([P, NODE_F], f32)
            nc.scalar.activation(out=t_r[:], in_=t_d[:], func=Act.Relu, bias=0.0, scale=1.0)
            t_sc = accp.tile([P, NODE_F], f32)
            nc.vector.tensor_tensor(out=t_sc[:], in0=t_r[:], in1=t_r[:], op=Alu.mult)
            t_ired = accp.tile([P, 1], f32)
            nc.vector.tensor_reduce(out=t_ired[:], in_=t_sc[:], axis=mybir.AxisListType.X, op=Alu.add)
            nc.vector.tensor_tensor(out=accI[:], in0=accI[:], in1=t_ired[:], op=Alu.add)

            # ---- element tiles ---------------------------------------------
            sizes = [16, 48] + [TE_] * ((EPP - 64) // TE_)
            assert sum(sizes) == EPP
            offs = [sum(sizes[:i]) for i in range(len(sizes))]
            for t, (eo, sz) in enumerate(zip(offs, sizes)):
                sl12 = slice(eo * 12, (eo + sz) * 12)
                sl16 = slice(eo * 16, (eo + sz) * 16)
                sl32 = slice(eo * 32, (eo + sz) * 32)
                sl96 = slice(eo * 96, (eo + sz) * 96)
                sl4 = slice(eo * 4, (eo + sz) * 4)

                t_uvc = loads.tile([P, sz * 12], f32)
                t_n = loads.tile([P, sz * 16], bf16)
                t_dn = loads.tile([P, sz * 32], bf16)
                t_b = loads.tile([P, sz * 96], bf16)
                t_vol = loads.tile([P, sz * 4], f32)
                nc.sync.dma_start(out=t_uvc[:], in_=d_uvc[:, sl12])
                nc.sync.dma_start(out=t_n[:], in_=d_n[:, sl16])
                nc.sync.dma_start(out=t_dn[:], in_=d_dn[:, sl32])
                nc.sync.dma_start(out=t_b[:], in_=d_b[:, sl96])
                nc.sync.dma_start(out=t_vol[:], in_=d_vol[:, sl4])

                if not COMPUTE:
                    continue
                uvc_r = t_uvc[:].rearrange("p (e n c) -> p e n c", n=4, c=3)
                n_r = t_n[:].rearrange("p (e i n) -> p e i n", i=4, n=4)
                dn_r = t_dn[:].rearrange("p (e x n) -> p e x n", x=8, n=4)
                b_r = t_b[:].rearrange("p (e y j) -> p e y j", y=12, j=8)
                vol_f = t_vol[:]  # [P, TE*4]

                # contiguous bf16 copy of c_el; broadcast views from it
                t_cel = scratch.tile([P, sz, 4], bf16)
                nc.scalar.copy(out=t_cel[:], in_=uvc_r[:, :, :, 2:3].squeeze(3))
                cel_ip = t_cel[:].unsqueeze(2).broadcast_to([P, sz, 4, 4])
                cel_g = t_cel[:].unsqueeze(2).broadcast_to([P, sz, 8, 4])

                # uv interleave [p e 8] <- uvc[..., 0:2]
                t_uv = scratch.tile([P, sz * 8], bf16)
                uv_w = t_uv[:].rearrange("p (e n c) -> p e n c", n=4, c=2)
                nc.scalar.copy(out=uv_w, in_=uvc_r[:, :, :, 0:2])
                uv_b = (
                    t_uv[:].rearrange("p (e j) -> p e j", j=8)
                    .unsqueeze(2).broadcast_to([P, sz, 12, 8])
                )

                # nc_ip = sum_n N * c_el  -> [p e i]
                t_m1 = scratch.tile([P, sz, 4, 4], bf16)
                nc.gpsimd.tensor_tensor(out=t_m1[:], in0=n_r, in1=cel_ip, op=Alu.mult)
                t_m1h = scratch.tile([P, sz, 4, 2], f32)
                nc.gpsimd.tensor_tensor(out=t_m1h[:], in0=t_m1[:, :, :, 0:2], in1=t_m1[:, :, :, 2:4], op=Alu.add)
                t_nc = scratch.tile([P, sz, 4], f32)
                nc.vector.tensor_tensor(out=t_nc[:], in0=t_m1h[:, :, :, 0:1].squeeze(3), in1=t_m1h[:, :, :, 1:2].squeeze(3), op=Alu.add)

                # grad = sum_n dNdx * c_el -> [p e i d]
                t_m2 = scratch.tile([P, sz, 8, 4], bf16)
                nc.gpsimd.tensor_tensor(out=t_m2[:], in0=dn_r, in1=cel_g, op=Alu.mult)
                t_m2h = scratch.tile([P, sz, 8, 2], f32)
                nc.gpsimd.tensor_tensor(out=t_m2h[:], in0=t_m2[:, :, :, 0:2], in1=t_m2[:, :, :, 2:4], op=Alu.add)
                t_gr = scratch.tile([P, sz, 4, 2], f32)
                nc.vector.tensor_tensor(out=t_gr[:].rearrange("p e i d -> p (e i d)").rearrange("p (x) -> p x"), in0=t_m2h[:, :, :, 0:1].squeeze(3).rearrange("p e x -> p (e x)"), in1=t_m2h[:, :, :, 1:2].squeeze(3).rearrange("p e x -> p (e x)"), op=Alu.add)

                # gsq = grad_x^2 + grad_y^2 -> [p e i]
                t_g2 = scratch.tile([P, sz, 4, 2], f32)
                nc.gpsimd.tensor_tensor(out=t_g2[:], in0=t_gr[:], in1=t_gr[:], op=Alu.mult)
                t_gs = scratch.tile([P, sz, 4], f32)
                nc.vector.tensor_reduce(out=t_gs[:], in_=t_g2[:], axis=mybir.AxisListType.X, op=Alu.add)

                # q = nc^2 + L0^2 * gsq ; E_frac partial += q * vol
                t_cs = scratch.tile([P, sz, 4], f32)
                nc.scalar.activation(out=t_cs[:], in_=t_nc[:], func=Act.Square, bias=0.0, scale=1.0)
                t_q = scratch.tile([P, sz, 4], f32)
                nc.scalar.activation(out=t_q[:], in_=t_gs[:], func=Act.Copy, bias=0.0, scale=float(L_0 * L_0))
                t_qt = scratch.tile([P, sz, 4], f32)
                nc.vector.tensor_tensor(out=t_qt[:], in0=t_q[:], in1=t_cs[:], op=Alu.add)
                t_s1 = scratch.tile([P, sz * 4], f32)
                nc.vector.tensor_tensor(out=t_s1[:], in0=t_qt[:].rearrange("p e i -> p (e i)"), in1=vol_f, op=Alu.mult)
                t_fred = scratch.tile([P, 1], f32)
                nc.vector.tensor_reduce(out=t_fred[:], in_=t_s1[:], axis=mybir.AxisListType.X, op=Alu.add)
                nc.vector.tensor_tensor(out=accF[:], in0=accF[:], in1=t_fred[:], op=Alu.add)

                # strain = sum_j B * uv -> [p e i k]
                t_m4 = scratch.tile([P, sz, 12, 8], bf16)
                nc.vector.tensor_tensor(out=t_m4[:, :, 0:8, :], in0=b_r[:, :, 0:8, :], in1=uv_b[:, :, 0:8, :], op=Alu.mult)
                nc.gpsimd.tensor_tensor(out=t_m4[:, :, 8:12, :], in0=b_r[:, :, 8:12, :], in1=uv_b[:, :, 8:12, :], op=Alu.mult)
                t_m4h = scratch.tile([P, sz, 12, 4], f32)
                nc.gpsimd.tensor_tensor(out=t_m4h[:], in0=t_m4[:, :, :, 0:4], in1=t_m4[:, :, :, 4:8], op=Alu.add)
                t_st = scratch.tile([P, sz, 4, 3], f32)
                nc.vector.tensor_reduce(out=t_st[:], in_=t_m4h[:].rearrange("p e y j -> p (e y) j"), axis=mybir.AxisListType.X, op=Alu.add)

                a_v = t_st[:, :, :, 0:1].squeeze(3)
                b_v = t_st[:, :, :, 1:2].squeeze(3)
                s2_v = t_st[:, :, :, 2:3].squeeze(3)

                # tr = a+b ; sd = a-b ; dev2 = tr^2/6 + sd^2/2 + s2^2/2
                t_tr = scratch.tile([P, sz, 4], f32)
                nc.vector.tensor_tensor(out=t_tr[:], in0=a_v, in1=b_v, op=Alu.add)
                t_sd = scratch.tile([P, sz, 4], f32)
                nc.vector.tensor_tensor(out=t_sd[:], in0=a_v, in1=b_v, op=Alu.subtract)
                t_rp = scratch.tile([P, sz, 4], f32)
                nc.scalar.activation(out=t_rp[:], in_=t_tr[:], func=Act.Relu, bias=0.0, scale=1.0)
                t_rps = scratch.tile([P, sz, 4], f32)
                nc.scalar.activation(out=t_rps[:], in_=t_rp[:], func=Act.Square, bias=0.0, scale=float((0.5 * K_MOD) ** 0.5))
                t_rns = scratch.tile([P, sz, 4], f32)
                nc.scalar.activation(out=t_rns[:], in_=t_tr[:], func=Act.Relu, bias=0.0, scale=-1.0)
                t_rnsq = scratch.tile([P, sz, 4], f32)
                nc.scalar.activation(out=t_rnsq[:], in_=t_rns[:], func=Act.Square, bias=0.0, scale=float((0.5 * K_MOD) ** 0.5))
                t_trs = scratch.tile([P, sz, 4], f32)
                nc.scalar.activation(out=t_trs[:], in_=t_tr[:], func=Act.Square, bias=0.0, scale=float((MU / 6.0) ** 0.5))
                t_sds = scratch.tile([P, sz, 4], f32)
                nc.scalar.activation(out=t_sds[:], in_=t_sd[:], func=Act.Square, bias=0.0, scale=float((0.5 * MU) ** 0.5))
                t_ss = scratch.tile([P, sz, 4], f32)
                nc.scalar.activation(out=t_ss[:], in_=s2_v, func=Act.Square, bias=0.0, scale=float((0.5 * MU) ** 0.5))

                # m = 1 - nc ; g = m^2
                t_mm = scratch.tile([P, sz, 4], f32)
                nc.scalar.activation(out=t_mm[:], in_=t_nc[:], func=Act.Copy, bias=1.0, scale=-1.0)
                t_gg = scratch.tile([P, sz, 4], f32)
                nc.scalar.activation(out=t_gg[:], in_=t_mm[:], func=Act.Square, bias=0.0, scale=1.0)

                # zp = 0.5K*rps + MU/6*trs + MU/2*sds + MU/2*ss ; psim = 0.5K*rnsq
                t_z12 = scratch.tile([P, sz, 4], f32)
                nc.gpsimd.tensor_tensor(out=t_z12[:], in0=t_rps[:], in1=t_trs[:], op=Alu.add)
                t_z34 = scratch.tile([P, sz, 4], f32)
                nc.gpsimd.tensor_tensor(out=t_z34[:], in0=t_sds[:], in1=t_ss[:], op=Alu.add)
                t_zp = scratch.tile([P, sz, 4], f32)
                nc.gpsimd.tensor_tensor(out=t_zp[:], in0=t_z12[:], in1=t_z34[:], op=Alu.add)
                t_zg = scratch.tile([P, sz, 4], f32)
                nc.vector.tensor_tensor(out=t_zg[:], in0=t_zp[:], in1=t_gg[:], op=Alu.mult)
                t_cb = scratch.tile([P, sz, 4], f32)
                nc.vector.tensor_tensor(out=t_cb[:], in0=t_zg[:], in1=t_rnsq[:], op=Alu.add)
                t_s2c = scratch.tile([P, sz * 4], f32)
                nc.vector.tensor_tensor(out=t_s2c[:], in0=t_cb[:].rearrange("p e i -> p (e i)"), in1=vol_f, op=Alu.mult)
                t_ered = scratch.tile([P, 1], f32)
                nc.vector.tensor_reduce(out=t_ered[:], in_=t_s2c[:], axis=mybir.AxisListType.X, op=Alu.add)
                nc.vector.tensor_tensor(out=accE[:], in0=accE[:], in1=t_ered[:], op=Alu.add)

            t_out = accp.tile([P, 4], f32)
            nc.vector.memset(t_out[:], 0.0)
            nc.vector.tensor_copy(out=t_out[:, 0:1], in_=accE[:])
            nc.vector.tensor_copy(out=t_out[:, 1:2], in_=accF[:])
            nc.vector.tensor_copy(out=t_out[:, 2:3], in_=accI[:])
            nc.sync.dma_start(out=d_out[:], in_=t_out[:])

    nc.compile()
    return nc


def kernel(u, v, c, prev_c, connectivities, N, dNdx, B, volumes):
    global LAST_EXEC_NS
    if "nc" not in _CACHE:
        _CACHE["nc"] = _build_bass()
    nc = _CACHE["nc"]
    from concourse.bass_utils import run_bass_kernel_spmd

    u = np.asarray(u, dtype=np.float32)
    v = np.asarray(v, dtype=np.float32)
    c = np.asarray(c, dtype=np.float32)
    prev_c = np.asarray(prev_c, dtype=np.float32)
    conn = np.asarray(connectivities)
    bf = ml_dtypes.bfloat16

    # node -> element-space layout prep (uvc triples per element corner)
    uvc_full = np.stack([u, v, c], axis=1)                 # [N_NODES, 3] f32
    uvc_el = uvc_full[conn.reshape(-1)].reshape(N_ELEMS, 12)
    n_bf = np.ascontiguousarray(N, dtype=np.float32).astype(bf).reshape(N_ELEMS, 16)
    dn_bf = np.ascontiguousarray(dNdx, dtype=np.float32).astype(bf).reshape(N_ELEMS, 32)
    b_bf = np.ascontiguousarray(B, dtype=np.float32).astype(bf).reshape(N_ELEMS, 96)
    vol = np.ascontiguousarray(volumes, dtype=np.float32).reshape(N_ELEMS, 4)

    c_pad = np.zeros(NODE_PAD * NCORES, np.float32)
    c_pad[:N_NODES] = c
    pc_pad = np.zeros(NODE_PAD * NCORES, np.float32)
    pc_pad[:N_NODES] = prev_c

    in_maps = []
    for i in range(NCORES):
        es = slice(i * EC, (i + 1) * EC)
        ns = slice(i * NODE_PAD, (i + 1) * NODE_PAD)
        in_maps.append({
            "uvc": uvc_el[es].reshape(P, EPP * 12),
            "nmat": n_bf[es].reshape(P, EPP * 16),
            "dmat": dn_bf[es].reshape(P, EPP * 32),
            "bmat": b_bf[es].reshape(P, EPP * 96),
            "vol": vol[es].reshape(P, EPP * 4),
            "cnd": c_pad[ns].reshape(P, NODE_F),
            "pnd": pc_pad[ns].reshape(P, NODE_F),
        })

    r = run_bass_kernel_spmd(nc, in_maps, core_ids=list(range(NCORES)), trace=TRACE)
    LAST_EXEC_NS = r.exec_time_ns

    parts = np.stack([np.asarray(r.results[i]["out"], dtype=np.float64) for i in range(NCORES)])
    sums = parts.sum(axis=(0, 1))                          # [4]
    e_el = sums[0]
    e_fr = (G_C / (2.0 * L_0)) * sums[1]
    e_ir = 0.5 * PENALTY * sums[2]
    return (np.float32(e_el), np.float32(e_fr), np.float32(e_ir))



# revision 3
# speedup vs baseline: 1.6314x; 1.6314x over previous
"""Trainium2 Bass kernel v2 for phase-field fracture FEM energy.

Per-core device kernel (32768 elems = 128 partitions x 256 elems), comp-major
tiles. Host folds volumes+constants into coefficient streams:

  E_frac  = sum_e Q[10] . cc[10]           (cc = upper-tri c (x) c products)
  t,d,g   = per-ip dots of fp8 strain rows with uv (Pool mults, DVE adds)
  psi-    = sum relu(-t)^2                 (Act relu + square-accum)
  E_el+   = sum (relu(t)^2 + rho t^2 + d^2 + g^2) * (1-s)^2
  s       = per-ip N . c                   (for the degradation factor)
  E_irr   = sum relu(prev_c - c)^2         (nodal shard)

Scalar sums are per-partition slots -> out [128, 8]; host does the final sum.
"""
import numpy as np

# --- problem constants (from reference) --------------------------------------
G_C = 0.0027
L_0 = 0.015
PF_TOL = 0.01
ENERGY_SCALING = 1.0
NU = 0.3
E_MOD = 210.0
LAM = E_MOD * NU / ((1.0 + NU) * (1.0 - 2.0 * NU))
MU = E_MOD / (2.0 * (1.0 + NU))
K_MOD = LAM + 2.0 * MU / 3.0
PENALTY = G_C / L_0 * (1.0 / PF_TOL**2 - 1.0) * ENERGY_SCALING
KF = G_C / (2.0 * L_0)
RHO = MU / (3.0 * K_MOD)          # (MU/6)/(K/2)

N_NODES = 263169
N_ELEMS = 262144
NCORES = 8
P = 128
EC = N_ELEMS // NCORES            # 32768 elements per core
EPP = EC // P                     # 256 elements per partition
SIZES = [8, 16, 32, 64, 64, 48, 24]   # per-partition tile sizes (sum = EPP)
assert sum(SIZES) == EPP
NT = len(SIZES)
K8 = 96                           # fp8 rows: St(4x8), Ss(4x8), Sg(4x8) kind-major
K16 = 38                          # fp16 rows: c 4, uv 8, N 16, Q 10
OFF_C, OFF_UV, OFF_N, OFF_Q = 0, 4, 12, 28
NODE_PAD = 33024                  # per-core node shard rows (128*258)
NODE_F = NODE_PAD // P            # 258

_CACHE = {}


def _build_bass():
    import concourse.bacc as bacc
    import concourse.tile as tile
    from concourse import mybir

    f32 = mybir.dt.float32
    f16 = mybir.dt.float16
    f8 = mybir.dt.float8e4
    Alu = mybir.AluOpType
    Act = mybir.ActivationFunctionType

    nc = bacc.Bacc("TRN2")
    d_s8 = nc.dram_tensor("s8", [P, EPP * K8], f8, kind="ExternalInput")
    d_s16 = nc.dram_tensor("s16", [P, EPP * K16], f16, kind="ExternalInput")
    d_c = nc.dram_tensor("cnd", [P, NODE_F], f16, kind="ExternalInput")
    d_pc = nc.dram_tensor("pnd", [P, NODE_F], f16, kind="ExternalInput")
    d_out = nc.dram_tensor("out", [P, 8], f32, kind="ExternalOutput")

    with tile.TileContext(nc) as tc:
        with (
            tc.tile_pool(name="loads", bufs=3) as loads,
            tc.tile_pool(name="scratch", bufs=3) as scratch,
            tc.tile_pool(name="acc", bufs=1) as accp,
        ):
            slotE = accp.tile([P, NT], f32)
            slotR = accp.tile([P, NT], f32)
            slotF = accp.tile([P, NT], f32)
            slotI = accp.tile([P, 1], f32)
            nc.vector.memset(slotE[:], 0.0)
            nc.vector.memset(slotR[:], 0.0)
            nc.vector.memset(slotF[:], 0.0)
            nc.vector.memset(slotI[:], 0.0)

            offs = [sum(SIZES[:i]) for i in range(NT)]
            for t, (eo, sz) in enumerate(zip(offs, SIZES)):
                t8 = loads.tile([P, K8 * sz], f8)
                t16 = loads.tile([P, K16 * sz], f16)
                nc.sync.dma_start(out=t8[:], in_=d_s8[:, eo * K8:(eo + sz) * K8])
                nc.scalar.dma_start(out=t16[:], in_=d_s16[:, eo * K16:(eo + sz) * K16])

                r8 = t8[:].rearrange("p (r j e) -> p r j e", r=12, j=8)
                r16 = t16[:].rearrange("p (k e) -> p k e", k=K16)
                c4 = r16[:, OFF_C:OFF_C + 4, :]                      # [P,4,sz]
                uv = r16[:, OFF_UV:OFF_UV + 8, :]                    # [P,8,sz]
                nmat = r16[:, OFF_N:OFF_N + 16, :].rearrange("p (i n) e -> p i n e", i=4)
                qmat = r16[:, OFF_Q:OFF_Q + 10, :]                   # [P,10,sz]

                uv_b = uv.unsqueeze(1).broadcast_to([P, 12, 8, sz])
                c_b = c4.unsqueeze(1).broadcast_to([P, 4, 4, sz])

                # ---- strain dots: Pool does the big fp8 mult -----------------
                pb = scratch.tile([P, 12, 8, sz], f16)
                nc.gpsimd.tensor_tensor(out=pb[:], in0=r8, in1=uv_b, op=Alu.mult)
                g1 = scratch.tile([P, 12, 4, sz], f16)
                nc.vector.tensor_tensor(out=g1[:], in0=pb[:, :, 0:4, :], in1=pb[:, :, 4:8, :], op=Alu.add)
                g2 = scratch.tile([P, 12, 2, sz], f16)
                nc.vector.tensor_tensor(out=g2[:], in0=g1[:, :, 0:2, :], in1=g1[:, :, 2:4, :], op=Alu.add)
                f2 = scratch.tile([P, 12, sz], f16)
                nc.vector.tensor_tensor(
                    out=f2[:].unsqueeze(2), in0=g2[:, :, 0:1, :], in1=g2[:, :, 1:2, :], op=Alu.add)

                # ---- fracture quadratic form --------------------------------
                cc = scratch.tile([P, 10, sz], f16)
                nc.scalar.activation(out=cc[:, 0:4, :], in_=c4, func=Act.Square,
                                     bias=0.0, scale=1.0)
                nc.gpsimd.tensor_tensor(out=cc[:, 4:7, :], in0=c4[:, 0:3, :], in1=c4[:, 1:4, :], op=Alu.mult)
                nc.gpsimd.tensor_tensor(out=cc[:, 7:9, :], in0=c4[:, 0:2, :], in1=c4[:, 2:4, :], op=Alu.mult)
                nc.gpsimd.tensor_tensor(out=cc[:, 9:10, :], in0=c4[:, 0:1, :], in1=c4[:, 3:4, :], op=Alu.mult)
                fr = scratch.tile([P, 10, sz], f16)
                nc.gpsimd.tensor_tensor(out=fr[:], in0=qmat, in1=cc[:], op=Alu.mult)
                frs = scratch.tile([P, 10, sz], f16)
                nc.scalar.activation(out=frs[:], in_=fr[:], func=Act.Copy,
                                     bias=0.0, scale=1.0, accum_out=slotF[:, t:t + 1])

                # ---- degradation m^2 = (s-1)^2 ------------------------------
                pn = scratch.tile([P, 4, 4, sz], f16)
                nc.vector.tensor_tensor(out=pn[:], in0=nmat, in1=c_b, op=Alu.mult)
                sn1 = scratch.tile([P, 4, 2, sz], f16)
                nc.vector.tensor_tensor(out=sn1[:], in0=pn[:, :, 0:2, :], in1=pn[:, :, 2:4, :], op=Alu.add)
                s4 = scratch.tile([P, 4, sz], f16)
                nc.vector.tensor_tensor(
                    out=s4[:].unsqueeze(2), in0=sn1[:, :, 0:1, :], in1=sn1[:, :, 1:2, :], op=Alu.add)
                mm = scratch.tile([P, 4, sz], f16)
                nc.vector.tensor_scalar(out=mm[:], in0=s4[:], scalar1=-1.0,
                                        scalar2=None, op0=Alu.add)
                m2 = scratch.tile([P, 4, sz], f16)
                nc.vector.tensor_tensor(out=m2[:], in0=mm[:], in1=mm[:], op=Alu.mult)

                # ---- elastic assembly ---------------------------------------
                # psi+ = relu(t)^2 + rho t^2 + d^2 + g^2 = (1+rho) t^2 - nsq + d^2 + g^2
                # psi- = relu(-t)^2 = nsq ;  nsq = min(t,0)^2
                trow = f2[:, 0:4, :]
                st = scratch.tile([P, 4, sz], f16)
                nc.vector.tensor_scalar(out=st[:], in0=trow, scalar1=float(np.sqrt(1.0 + RHO)),
                                        scalar2=None, op0=Alu.mult)
                t2s = scratch.tile([P, 4, sz], f16)
                nc.vector.tensor_tensor(out=t2s[:], in0=st[:], in1=st[:], op=Alu.mult)
                n1 = scratch.tile([P, 4, sz], f16)
                nc.vector.tensor_scalar(out=n1[:], in0=trow, scalar1=0.0,
                                        scalar2=None, op0=Alu.min)
                nsq = scratch.tile([P, 4, sz], f16)
                nc.vector.tensor_tensor(out=nsq[:], in0=n1[:], in1=n1[:], op=Alu.mult)
                dsq = scratch.tile([P, 8, sz], f16)
                nc.scalar.activation(out=dsq[:], in_=f2[:, 4:12, :], func=Act.Square,
                                     bias=0.0, scale=1.0)
                dg = scratch.tile([P, 4, sz], f16)
                nc.gpsimd.tensor_tensor(out=dg[:], in0=dsq[:, 0:4, :], in1=dsq[:, 4:8, :], op=Alu.add)
                pa = scratch.tile([P, 4, sz], f16)
                nc.vector.tensor_tensor(out=pa[:], in0=t2s[:], in1=nsq[:], op=Alu.subtract)
                psi = scratch.tile([P, 4, sz], f16)
                nc.vector.tensor_tensor(out=psi[:], in0=pa[:], in1=dg[:], op=Alu.add)
                el = scratch.tile([P, 4, sz], f16)
                nc.vector.tensor_tensor(out=el[:], in0=psi[:], in1=m2[:], op=Alu.mult)
                el2 = scratch.tile([P, 4, sz], f16)
                nc.vector.tensor_tensor(out=el2[:], in0=el[:], in1=nsq[:], op=Alu.add)
                els = scratch.tile([P, 4, sz], f16)
                nc.scalar.activation(out=els[:], in_=el2[:], func=Act.Copy,
                                     bias=0.0, scale=1.0, accum_out=slotE[:, t:t + 1])

            # ---- E_irr over the node shard ----------------------------------
            t_c = accp.tile([P, NODE_F], f16)
            t_pc = accp.tile([P, NODE_F], f16)
            nc.sync.dma_start(out=t_c[:], in_=d_c[:])
            nc.sync.dma_start(out=t_pc[:], in_=d_pc[:])
            t_d = accp.tile([P, NODE_F], f16)
            nc.vector.tensor_tensor(out=t_d[:], in0=t_pc[:], in1=t_c[:], op=Alu.subtract)
            t_r = accp.tile([P, NODE_F], f16)
            nc.vector.tensor_scalar(out=t_r[:], in0=t_d[:], scalar1=0.0,
                                    scalar2=None, op0=Alu.max)
            t_rs = accp.tile([P, NODE_F], f16)
            nc.scalar.activation(out=t_rs[:], in_=t_r[:], func=Act.Square,
                                 bias=0.0, scale=1.0, accum_out=slotI[:])

            # ---- fold slots -> out ------------------------------------------
            t_out = accp.tile([P, 8], f32)
            nc.vector.memset(t_out[:], 0.0)
            nc.vector.tensor_reduce(out=t_out[:, 0:1], in_=slotE[:], axis=mybir.AxisListType.X, op=Alu.add)
            nc.vector.tensor_reduce(out=t_out[:, 1:2], in_=slotR[:], axis=mybir.AxisListType.X, op=Alu.add)
            nc.vector.tensor_reduce(out=t_out[:, 2:3], in_=slotF[:], axis=mybir.AxisListType.X, op=Alu.add)
            nc.vector.tensor_copy(out=t_out[:, 3:4], in_=slotI[:])
            nc.sync.dma_start(out=d_out[:], in_=t_out[:])

    nc.compile()
    return nc


def _host_prep(u, v, c, prev_c, connectivities, N, dNdx, B, volumes):
    from concourse import mybir
    f8np = mybir.dt.np(mybir.dt.float8e4)

    conn = np.asarray(connectivities)
    c = np.asarray(c, np.float32)
    u = np.asarray(u, np.float32)
    v = np.asarray(v, np.float32)
    prev_c = np.asarray(prev_c, np.float32)
    N = np.asarray(N, np.float32)
    dNdx = np.asarray(dNdx, np.float32)
    B = np.asarray(B, np.float32)
    w = np.asarray(volumes, np.float32)                    # [E,4]

    c_el = c[conn]                                         # [E,4]
    u_el = u[conn]
    v_el = v[conn]
    uv = np.empty((N_ELEMS, 8), np.float32)
    uv[:, 0::2] = u_el
    uv[:, 1::2] = v_el

    # strain coefficient rows, scaled so energies are plain sums of squares
    st = np.sqrt(0.5 * K_MOD * w)[..., None] * (B[:, :, 0, :] + B[:, :, 1, :])   # [E,4,8]
    ss = np.sqrt(0.5 * MU * w)[..., None] * (B[:, :, 0, :] - B[:, :, 1, :])
    sg = np.sqrt(0.5 * MU * w)[..., None] * B[:, :, 2, :]
    s8 = np.concatenate([st, ss, sg], axis=1).reshape(N_ELEMS, K8)               # kind-major [12,8]

    # fracture quadratic form: Q = sum_i kf*w_i*(N_i N_i^T + L0^2 D D^T)
    qf = np.einsum('ei,ein,eim->enm', KF * w, N, N)
    qf += np.einsum('ei,eidn,eidm->enm', KF * L_0 * L_0 * w, dNdx, dNdx)
    iu = [(0, 0), (1, 1), (2, 2), (3, 3), (0, 1), (1, 2), (2, 3), (0, 2), (1, 3), (0, 3)]
    q10 = np.stack([qf[:, i, j] * (1.0 if i == j else 2.0) for i, j in iu], axis=1)  # [E,10]

    nraw = N.reshape(N_ELEMS, 16)                          # ip-major [4,4]

    s16 = np.concatenate([c_el, uv, nraw, q10], axis=1)    # [E,38]
    assert s16.shape[1] == K16

    # comp-major variable-size tile blocks: [P, sum_t(K*sz)] per core
    def pack(arr, K, dtype):
        a = arr.reshape(NCORES, P, EPP, K)
        out = np.empty((NCORES, P, EPP * K), dtype)
        offs = np.cumsum([0] + SIZES)
        pos = 0
        for t, sz in enumerate(SIZES):
            blk = a[:, :, offs[t]:offs[t + 1], :]          # [NC,P,sz,K]
            out[:, :, pos:pos + K * sz] = (
                blk.transpose(0, 1, 3, 2).reshape(NCORES, P, K * sz).astype(dtype))
            pos += K * sz
        return out

    s8p = pack(s8, K8, f8np)
    s16p = pack(s16, K16, np.float16)

    c_pad = np.zeros(NODE_PAD * NCORES, np.float16)
    c_pad[:N_NODES] = c.astype(np.float16)
    pc_pad = np.zeros(NODE_PAD * NCORES, np.float16)
    pc_pad[:N_NODES] = prev_c.astype(np.float16)

    in_maps = []
    for i in range(NCORES):
        ns = slice(i * NODE_PAD, (i + 1) * NODE_PAD)
        in_maps.append({
            "s8": s8p[i],
            "s16": s16p[i],
            "cnd": c_pad[ns].reshape(P, NODE_F),
            "pnd": pc_pad[ns].reshape(P, NODE_F),
        })
    return in_maps


def kernel(u, v, c, prev_c, connectivities, N, dNdx, B, volumes):
    if "nc" not in _CACHE:
        _CACHE["nc"] = _build_bass()
    nc = _CACHE["nc"]
    from concourse.bass_utils import run_bass_kernel_spmd

    in_maps = _host_prep(u, v, c, prev_c, connectivities, N, dNdx, B, volumes)
    r = run_bass_kernel_spmd(nc, in_maps, core_ids=list(range(NCORES)))

    parts = np.stack([np.asarray(r.results[i]["out"], dtype=np.float64) for i in range(NCORES)])
    sums = parts.sum(axis=(0, 1))                          # [8]
    e_el = sums[0] + sums[1]
    e_fr = sums[2]
    e_ir = 0.5 * PENALTY * sums[3]
    return (np.float32(e_el), np.float32(e_fr), np.float32(e_ir))


def predicted_exec_ns():
    """CoreSim cost-model exec time for one core (timing-only)."""
    if "nc" not in _CACHE:
        _CACHE["nc"] = _build_bass()
    from concourse.bass_interp import CoreSim
    sim = CoreSim(_CACHE["nc"], no_exec=True, publish_trace=False)
    sim.simulate()
    return sim.time


# revision 13
# speedup vs baseline: 1.6569x; 1.0156x over previous
"""Trainium2 Bass kernel v2 for phase-field fracture FEM energy.

Per-core device kernel (32768 elems = 128 partitions x 256 elems), comp-major
tiles. Host folds volumes+constants into coefficient streams:

  E_frac  = sum_e Q[10] . cc[10]           (cc = upper-tri c (x) c products)
  t,d,g   = per-ip dots of fp8 strain rows with uv (Pool mults, DVE adds)
  psi-    = sum relu(-t)^2                 (Act relu + square-accum)
  E_el+   = sum (relu(t)^2 + rho t^2 + d^2 + g^2) * (1-s)^2
  s       = per-ip N . c                   (for the degradation factor)
  E_irr   = sum relu(prev_c - c)^2         (nodal shard)

Scalar sums are per-partition slots -> out [128, 8]; host does the final sum.
"""
import numpy as np

# --- problem constants (from reference) --------------------------------------
G_C = 0.0027
L_0 = 0.015
PF_TOL = 0.01
ENERGY_SCALING = 1.0
NU = 0.3
E_MOD = 210.0
LAM = E_MOD * NU / ((1.0 + NU) * (1.0 - 2.0 * NU))
MU = E_MOD / (2.0 * (1.0 + NU))
K_MOD = LAM + 2.0 * MU / 3.0
PENALTY = G_C / L_0 * (1.0 / PF_TOL**2 - 1.0) * ENERGY_SCALING
KF = G_C / (2.0 * L_0)
RHO = MU / (3.0 * K_MOD)          # (MU/6)/(K/2)

N_NODES = 263169
N_ELEMS = 262144
NCORES = 8
P = 128
EC = N_ELEMS // NCORES            # 32768 elements per core
EPP = EC // P                     # 256 elements per partition
SIZES = [8, 16, 32, 64, 64, 48, 24]  # per-partition tile sizes (sum = EPP)
assert sum(SIZES) == EPP
NT = len(SIZES)
K8 = 96                           # fp8 rows: St(4x8), Ss(4x8), Sg(4x8) kind-major
K16 = 38                          # fp16 rows: c 4, uv 8, N 16, Q 10
OFF_C, OFF_UV, OFF_N, OFF_Q = 0, 4, 12, 28
NODE_PAD = 33024                  # per-core node shard rows (128*258)
NODE_F = NODE_PAD // P            # 258

PB_SPLIT = False
_CACHE = {}


def _build_bass():
    import concourse.bacc as bacc
    import concourse.tile as tile
    from concourse import mybir

    f32 = mybir.dt.float32
    f16 = mybir.dt.float16
    f8 = mybir.dt.float8e4
    Alu = mybir.AluOpType
    Act = mybir.ActivationFunctionType

    nc = bacc.Bacc("TRN2")
    d_s8 = nc.dram_tensor("s8", [P, EPP * K8], f8, kind="ExternalInput")
    d_s16 = nc.dram_tensor("s16", [P, EPP * K16], f16, kind="ExternalInput")
    d_c = nc.dram_tensor("cnd", [P, NODE_F], f16, kind="ExternalInput")
    d_pc = nc.dram_tensor("pnd", [P, NODE_F], f16, kind="ExternalInput")
    d_out = nc.dram_tensor("out", [P, 8], f32, kind="ExternalOutput")

    with tile.TileContext(nc) as tc:
        with (
            tc.tile_pool(name="loads", bufs=4) as loads,
            tc.tile_pool(name="scratch", bufs=4) as scratch,
            tc.tile_pool(name="acc", bufs=1) as accp,
        ):
            slotE = accp.tile([P, NT], f32)
            slotR = accp.tile([P, NT], f32)
            slotF = accp.tile([P, NT], f32)
            slotI = accp.tile([P, 1], f32)
            nc.vector.memset(slotE[:], 0.0)
            nc.vector.memset(slotR[:], 0.0)
            nc.vector.memset(slotF[:], 0.0)
            nc.vector.memset(slotI[:], 0.0)

            offs = [sum(SIZES[:i]) for i in range(NT)]
            for t, (eo, sz) in enumerate(zip(offs, SIZES)):
                t8 = loads.tile([P, K8 * sz], f8)
                t16 = loads.tile([P, K16 * sz], f16)
                nc.sync.dma_start(out=t8[:], in_=d_s8[:, eo * K8:(eo + sz) * K8])
                nc.scalar.dma_start(out=t16[:], in_=d_s16[:, eo * K16:(eo + sz) * K16])

                r8 = t8[:].rearrange("p (r j e) -> p r j e", r=12, j=8)
                r16 = t16[:].rearrange("p (k e) -> p k e", k=K16)
                c4 = r16[:, OFF_C:OFF_C + 4, :]                      # [P,4,sz]
                uv = r16[:, OFF_UV:OFF_UV + 8, :]                    # [P,8,sz]
                nmat = r16[:, OFF_N:OFF_N + 16, :].rearrange("p (i n) e -> p i n e", i=4)
                qmat = r16[:, OFF_Q:OFF_Q + 10, :]                   # [P,10,sz]

                uv_b = uv.unsqueeze(1).broadcast_to([P, 12, 8, sz])
                c_b = c4.unsqueeze(1).broadcast_to([P, 4, 4, sz])

                # ---- strain dots: Pool fp8 mult (split t | d,g), DVE add-trees
                f2 = scratch.tile([P, 12, sz], f16)
                if PB_SPLIT:
                    pbA = scratch.tile([P, 4, 8, sz], f16)
                    nc.gpsimd.tensor_tensor(out=pbA[:], in0=r8[:, 0:4, :, :],
                                            in1=uv_b[:, 0:4, :, :], op=Alu.mult)
                    g1A = scratch.tile([P, 4, 4, sz], f16)
                    nc.vector.tensor_tensor(out=g1A[:], in0=pbA[:, :, 0:4, :], in1=pbA[:, :, 4:8, :], op=Alu.add)
                    g2A = scratch.tile([P, 4, 2, sz], f16)
                    nc.vector.tensor_tensor(out=g2A[:], in0=g1A[:, :, 0:2, :], in1=g1A[:, :, 2:4, :], op=Alu.add)
                    nc.vector.tensor_tensor(out=f2[:, 0:4, :].unsqueeze(2),
                                            in0=g2A[:, :, 0:1, :], in1=g2A[:, :, 1:2, :], op=Alu.add)
                    pbB = scratch.tile([P, 8, 8, sz], f16)
                    nc.gpsimd.tensor_tensor(out=pbB[:], in0=r8[:, 4:12, :, :],
                                            in1=uv_b[:, 4:12, :, :], op=Alu.mult)
                    g1B = scratch.tile([P, 8, 4, sz], f16)
                    nc.vector.tensor_tensor(out=g1B[:], in0=pbB[:, :, 0:4, :], in1=pbB[:, :, 4:8, :], op=Alu.add)
                    g2B = scratch.tile([P, 8, 2, sz], f16)
                    nc.vector.tensor_tensor(out=g2B[:], in0=g1B[:, :, 0:2, :], in1=g1B[:, :, 2:4, :], op=Alu.add)
                    nc.vector.tensor_tensor(out=f2[:, 4:12, :].unsqueeze(2),
                                            in0=g2B[:, :, 0:1, :], in1=g2B[:, :, 1:2, :], op=Alu.add)
                else:
                    pb = scratch.tile([P, 12, 8, sz], f16)
                    nc.gpsimd.tensor_tensor(out=pb[:], in0=r8, in1=uv_b, op=Alu.mult)
                    g1 = scratch.tile([P, 12, 4, sz], f16)
                    nc.vector.tensor_tensor(out=g1[:], in0=pb[:, :, 0:4, :], in1=pb[:, :, 4:8, :], op=Alu.add)
                    g2 = scratch.tile([P, 12, 2, sz], f16)
                    nc.vector.tensor_tensor(out=g2[:], in0=g1[:, :, 0:2, :], in1=g1[:, :, 2:4, :], op=Alu.add)
                    nc.vector.tensor_tensor(
                        out=f2[:].unsqueeze(2), in0=g2[:, :, 0:1, :], in1=g2[:, :, 1:2, :], op=Alu.add)

                # ---- fracture quadratic form --------------------------------
                cc = scratch.tile([P, 10, sz], f16)
                nc.scalar.activation(out=cc[:, 0:4, :], in_=c4, func=Act.Square,
                                     bias=0.0, scale=1.0)
                nc.gpsimd.tensor_tensor(out=cc[:, 4:7, :], in0=c4[:, 0:3, :], in1=c4[:, 1:4, :], op=Alu.mult)
                nc.gpsimd.tensor_tensor(out=cc[:, 7:9, :], in0=c4[:, 0:2, :], in1=c4[:, 2:4, :], op=Alu.mult)
                nc.gpsimd.tensor_tensor(out=cc[:, 9:10, :], in0=c4[:, 0:1, :], in1=c4[:, 3:4, :], op=Alu.mult)
                fr = scratch.tile([P, 10, sz], f16)
                nc.gpsimd.tensor_tensor(out=fr[:], in0=qmat, in1=cc[:], op=Alu.mult)
                frs = scratch.tile([P, 10, sz], f16)
                nc.scalar.activation(out=frs[:], in_=fr[:], func=Act.Copy,
                                     bias=0.0, scale=1.0, accum_out=slotF[:, t:t + 1])

                # ---- degradation m^2 = (s-1)^2 ------------------------------
                pn = scratch.tile([P, 4, 4, sz], f16)
                nc.vector.tensor_tensor(out=pn[:], in0=nmat, in1=c_b, op=Alu.mult)
                sn1 = scratch.tile([P, 4, 2, sz], f16)
                nc.vector.tensor_tensor(out=sn1[:], in0=pn[:, :, 0:2, :], in1=pn[:, :, 2:4, :], op=Alu.add)
                s4 = scratch.tile([P, 4, sz], f16)
                nc.vector.tensor_tensor(
                    out=s4[:].unsqueeze(2), in0=sn1[:, :, 0:1, :], in1=sn1[:, :, 1:2, :], op=Alu.add)
                mm = scratch.tile([P, 4, sz], f16)
                nc.vector.tensor_scalar(out=mm[:], in0=s4[:], scalar1=-1.0,
                                        scalar2=None, op0=Alu.add)
                m2 = scratch.tile([P, 4, sz], f16)
                nc.vector.tensor_tensor(out=m2[:], in0=mm[:], in1=mm[:], op=Alu.mult)

                # ---- elastic assembly ---------------------------------------
                # psi+ = relu(t)^2 + rho t^2 + d^2 + g^2 = (1+rho) t^2 - nsq + d^2 + g^2
                # psi- = relu(-t)^2 = nsq ;  nsq = min(t,0)^2
                # host pre-scales St rows by sqrt(1+rho): trow = sqrt(1+rho)*t
                trow = f2[:, 0:4, :]
                t2s = scratch.tile([P, 4, sz], f16)
                nc.vector.tensor_tensor(out=t2s[:], in0=trow, in1=trow, op=Alu.mult)
                n1 = scratch.tile([P, 4, sz], f16)
                nc.vector.tensor_scalar(out=n1[:], in0=trow, scalar1=0.0,
                                        scalar2=float(1.0 / np.sqrt(1.0 + RHO)),
                                        op0=Alu.min, op1=Alu.mult)
                nsq = scratch.tile([P, 4, sz], f16)
                nc.vector.tensor_tensor(out=nsq[:], in0=n1[:], in1=n1[:], op=Alu.mult)
                dsq = scratch.tile([P, 8, sz], f16)
                nc.scalar.activation(out=dsq[:], in_=f2[:, 4:12, :], func=Act.Square,
                                     bias=0.0, scale=1.0)
                dg = scratch.tile([P, 4, sz], f16)
                nc.gpsimd.tensor_tensor(out=dg[:], in0=dsq[:, 0:4, :], in1=dsq[:, 4:8, :], op=Alu.add)
                pa = scratch.tile([P, 4, sz], f16)
                nc.vector.tensor_tensor(out=pa[:], in0=t2s[:], in1=nsq[:], op=Alu.subtract)
                psi = scratch.tile([P, 4, sz], f16)
                nc.vector.tensor_tensor(out=psi[:], in0=pa[:], in1=dg[:], op=Alu.add)
                el = scratch.tile([P, 4, sz], f16)
                nc.vector.tensor_tensor(out=el[:], in0=psi[:], in1=m2[:], op=Alu.mult)
                el2 = scratch.tile([P, 4, sz], f16)
                nc.vector.tensor_tensor(out=el2[:], in0=el[:], in1=nsq[:], op=Alu.add)
                els = scratch.tile([P, 4, sz], f16)
                nc.scalar.activation(out=els[:], in_=el2[:], func=Act.Copy,
                                     bias=0.0, scale=1.0, accum_out=slotE[:, t:t + 1])

            # ---- E_irr over the node shard ----------------------------------
            t_c = accp.tile([P, NODE_F], f16)
            t_pc = accp.tile([P, NODE_F], f16)
            nc.sync.dma_start(out=t_c[:], in_=d_c[:])
            nc.sync.dma_start(out=t_pc[:], in_=d_pc[:])
            t_d = accp.tile([P, NODE_F], f16)
            nc.vector.tensor_tensor(out=t_d[:], in0=t_pc[:], in1=t_c[:], op=Alu.subtract)
            t_r = accp.tile([P, NODE_F], f16)
            nc.vector.tensor_scalar(out=t_r[:], in0=t_d[:], scalar1=0.0,
                                    scalar2=None, op0=Alu.max)
            t_rs = accp.tile([P, NODE_F], f16)
            nc.scalar.activation(out=t_rs[:], in_=t_r[:], func=Act.Square,
                                 bias=0.0, scale=1.0, accum_out=slotI[:])

            # ---- fold slots -> out ------------------------------------------
            t_out = accp.tile([P, 8], f32)
            nc.vector.memset(t_out[:], 0.0)
            nc.vector.tensor_reduce(out=t_out[:, 0:1], in_=slotE[:], axis=mybir.AxisListType.X, op=Alu.add)
            nc.vector.tensor_reduce(out=t_out[:, 1:2], in_=slotR[:], axis=mybir.AxisListType.X, op=Alu.add)
            nc.vector.tensor_reduce(out=t_out[:, 2:3], in_=slotF[:], axis=mybir.AxisListType.X, op=Alu.add)
            nc.vector.tensor_copy(out=t_out[:, 3:4], in_=slotI[:])
            nc.sync.dma_start(out=d_out[:], in_=t_out[:])

    nc.compile()
    return nc


def _host_prep(u, v, c, prev_c, connectivities, N, dNdx, B, volumes):
    from concourse import mybir
    f8np = mybir.dt.np(mybir.dt.float8e4)

    conn = np.asarray(connectivities)
    c = np.asarray(c, np.float32)
    u = np.asarray(u, np.float32)
    v = np.asarray(v, np.float32)
    prev_c = np.asarray(prev_c, np.float32)
    N = np.asarray(N, np.float32)
    dNdx = np.asarray(dNdx, np.float32)
    B = np.asarray(B, np.float32)
    w = np.asarray(volumes, np.float32)                    # [E,4]

    c_el = c[conn]                                         # [E,4]
    u_el = u[conn]
    v_el = v[conn]
    uv = np.empty((N_ELEMS, 8), np.float32)
    uv[:, 0::2] = u_el
    uv[:, 1::2] = v_el

    # strain coefficient rows, scaled so energies are plain sums of squares
    # (St additionally carries sqrt(1+rho) so t^2 on device is (1+rho)t^2)
    st = np.sqrt((1.0 + RHO) * 0.5 * K_MOD * w)[..., None] * (B[:, :, 0, :] + B[:, :, 1, :])   # [E,4,8]
    ss = np.sqrt(0.5 * MU * w)[..., None] * (B[:, :, 0, :] - B[:, :, 1, :])
    sg = np.sqrt(0.5 * MU * w)[..., None] * B[:, :, 2, :]
    s8 = np.concatenate([st, ss, sg], axis=1).reshape(N_ELEMS, K8)               # kind-major [12,8]

    # fracture quadratic form: Q = sum_i kf*w_i*(N_i N_i^T + L0^2 D D^T)
    qf = np.einsum('ei,ein,eim->enm', KF * w, N, N)
    qf += np.einsum('ei,eidn,eidm->enm', KF * L_0 * L_0 * w, dNdx, dNdx)
    iu = [(0, 0), (1, 1), (2, 2), (3, 3), (0, 1), (1, 2), (2, 3), (0, 2), (1, 3), (0, 3)]
    q10 = np.stack([qf[:, i, j] * (1.0 if i == j else 2.0) for i, j in iu], axis=1)  # [E,10]

    nraw = N.reshape(N_ELEMS, 16)                          # ip-major [4,4]

    s16 = np.concatenate([c_el, uv, nraw, q10], axis=1)    # [E,38]
    assert s16.shape[1] == K16

    # comp-major variable-size tile blocks: [P, sum_t(K*sz)] per core
    def pack(arr, K, dtype):
        a = arr.reshape(NCORES, P, EPP, K)
        out = np.empty((NCORES, P, EPP * K), dtype)
        offs = np.cumsum([0] + SIZES)
        pos = 0
        for t, sz in enumerate(SIZES):
            blk = a[:, :, offs[t]:offs[t + 1], :]          # [NC,P,sz,K]
            out[:, :, pos:pos + K * sz] = (
                blk.transpose(0, 1, 3, 2).reshape(NCORES, P, K * sz).astype(dtype))
            pos += K * sz
        return out

    s8p = pack(s8, K8, f8np)
    s16p = pack(s16, K16, np.float16)

    c_pad = np.zeros(NODE_PAD * NCORES, np.float16)
    c_pad[:N_NODES] = c.astype(np.float16)
    pc_pad = np.zeros(NODE_PAD * NCORES, np.float16)
    pc_pad[:N_NODES] = prev_c.astype(np.float16)

    in_maps = []
    for i in range(NCORES):
        ns = slice(i * NODE_PAD, (i + 1) * NODE_PAD)
        in_maps.append({
            "s8": s8p[i],
            "s16": s16p[i],
            "cnd": c_pad[ns].reshape(P, NODE_F),
            "pnd": pc_pad[ns].reshape(P, NODE_F),
        })
    return in_maps


def kernel(u, v, c, prev_c, connectivities, N, dNdx, B, volumes):
    if "nc" not in _CACHE:
        _CACHE["nc"] = _build_bass()
    nc = _CACHE["nc"]
    from concourse.bass_utils import run_bass_kernel_spmd

    in_maps = _host_prep(u, v, c, prev_c, connectivities, N, dNdx, B, volumes)
    r = run_bass_kernel_spmd(nc, in_maps, core_ids=list(range(NCORES)))

    parts = np.stack([np.asarray(r.results[i]["out"], dtype=np.float64) for i in range(NCORES)])
    sums = parts.sum(axis=(0, 1))                          # [8]
    e_el = sums[0] + sums[1]
    e_fr = sums[2]
    e_ir = 0.5 * PENALTY * sums[3]
    return (np.float32(e_el), np.float32(e_fr), np.float32(e_ir))


def predicted_exec_ns():
    """CoreSim cost-model exec time for one core (timing-only)."""
    if "nc" not in _CACHE:
        _CACHE["nc"] = _build_bass()
    from concourse.bass_interp import CoreSim
    sim = CoreSim(_CACHE["nc"], no_exec=True, publish_trace=False)
    sim.simulate()
    return sim.time
